# revision 27
# baseline (speedup 1.0000x reference)
"""Trainium2 Bass kernel: batched multi-head cross-attention.

Reference computation (per batch element b):
    q = x @ Wq; k,v = split(context @ Wkv)
    per head: attn = softmax(q k^T / 8); o = attn @ v
    out = concat_heads(o) @ Wo + bo

Sharding: pure data parallel - batch B=8, one batch element per NeuronCore,
no collectives. Inside each core a fully "transposed" dataflow (host feeds
x^T / context^T) avoids all on-device transposes:

    QT[d,i]  = Wq^T  @ xT
    KT[d,j]  = Wk^T  @ cT
    V[j,d]   = cT^T  @ Wv          (with a fused ones-column so the attn@V
                                    matmul also emits the softmax denominator)
    ST[j,i]  = KT_h^T @ QT_h       (per head, contraction d=64)
    PT       = exp(ST / 8)         (scores are O(6) so exp is fp32-safe and
                                    softmax is shift-invariant)
    OunT,l   = [V_h | 1]^T @ PT
    OnT      = OunT * (1/l)
    outT     = Wo^T @ OnT + bo

Scheduling: the PE HAM clock gate throttles the tensor engine to 1.2 GHz
whenever it sees idle windows, and the naive phase-separated schedule lets
the S->exp->O dependency chain starve the PE during attention (then the
half-clock PE *looks* saturated and never re-warms). To keep the PE stream
dense end-to-end, the QT/KT projections for later head-chunks and the
output-projection accumulation are emitted as *filler* matmul work inside
the attention loop; the PE's 64-deep reorder window picks whichever queued
matmul is ready. Out-projection partials accumulate into SBUF across head
pairs so PSUM stays within 8 banks:
    psS [128,1024] x2 (4 banks) + psO [65,1024] x1 (2) + paux [128,512] x2 (2).
"""

import numpy as np
import ml_dtypes

B, N, M, D = 8, 1024, 1024, 512
H, DH = 8, 64
KC = 4          # 512 contraction -> 4 chunks of 128
IC = 2          # 1024 free dim -> 2 chunks of 512
JC = 8          # 1024 keys -> 8 chunks of 128
EC = 4          # 512 output dim -> 4 chunks of 128
N_CORES = 8

_BF16 = ml_dtypes.bfloat16
_CACHE = {}
LAST_RUN = None  # BassKernelResults of the most recent launch (for test.py)


def _build_nc():
    import concourse.bass as bass
    import concourse.mybir as mybir
    import concourse.tile as tile
    from concourse import bacc

    f32 = mybir.dt.float32
    bf16 = mybir.dt.bfloat16
    Exp = mybir.ActivationFunctionType.Exp

    nc = bacc.Bacc()

    xt = nc.declare_dram_parameter("xt", [D, N], bf16, isOutput=False)
    ct = nc.declare_dram_parameter("ct", [D, M], bf16, isOutput=False)
    wq = nc.declare_dram_parameter("wq", [D, D], bf16, isOutput=False)
    wk = nc.declare_dram_parameter("wk", [D, D], bf16, isOutput=False)
    wv = nc.declare_dram_parameter("wv", [D, D], bf16, isOutput=False)
    wo = nc.declare_dram_parameter("wo", [D, D], bf16, isOutput=False)
    bo = nc.declare_dram_parameter("bo", [D, 1], f32, isOutput=False)
    outT = nc.declare_dram_parameter("outT", [D, N], f32, isOutput=True)

    with tile.TileContext(nc) as tc:
        with (
            tc.tile_pool(name="singles", bufs=1) as singles,
            tc.tile_pool(name="pt", bufs=6) as ptp,
            tc.tile_pool(name="of", bufs=2) as ofp,
            tc.tile_pool(name="lv", bufs=2) as lp,
            tc.tile_pool(name="rb", bufs=2) as rp,
            tc.tile_pool(name="ot", bufs=3) as otp,
            tc.tile_pool(name="paux", bufs=2, space="PSUM") as paux,
        ):
            def sb(shape, dt, tag):
                return singles.tile(shape, dt, tag=tag, name=tag)

            wq_sb = [sb([128, D], bf16, f"wq{c}") for c in range(KC)]
            xt_sb = [sb([128, N], bf16, f"xt{c}") for c in range(KC)]
            wk_sb = [sb([128, D], bf16, f"wk{c}") for c in range(KC)]
            ct_sb = [sb([128, M], bf16, f"ct{c}") for c in range(KC)]
            wv_sb = [sb([128, D], bf16, f"wv{c}") for c in range(KC)]
            wo_sb = [sb([128, D], bf16, f"wo{c}") for c in range(KC)]
            qt_sb = [sb([128, N], bf16, f"qt{c}") for c in range(KC)]
            kt_sb = [sb([128, M], bf16, f"kt{c}") for c in range(KC)]
            v_sb = [sb([128, H, DH + 1], bf16, f"v{j}") for j in range(JC)]
            o_sb = [sb([128, N], bf16, f"o{c}") for c in range(KC)]
            on_sb = [sb([128, N], bf16, f"on{c}") for c in range(KC)]
            acc_sb = [sb([128, N], f32, f"acc{e}") for e in range(EC)]
            acc2_sb = [sb([128, N], f32, f"ac2{e}") for e in range(EC)]
            bo_sb = sb([128, KC, 1], f32, "bo")

            # ---- loads: two HWDGE queues (sync + scalar), batches
            # ordered by when the pre-phase needs each tensor ----
            for c in range(KC):
                nc.sync.dma_start(out=wq_sb[c], in_=wq[c * 128:(c + 1) * 128, :])
                nc.scalar.dma_start(
                    out=xt_sb[c][:, 0:512], in_=xt[c * 128:(c + 1) * 128, 0:512]
                )
            for c in range(KC):
                nc.sync.dma_start(out=wk_sb[c], in_=wk[c * 128:(c + 1) * 128, :])
                nc.scalar.dma_start(
                    out=ct_sb[c][:, 0:512], in_=ct[c * 128:(c + 1) * 128, 0:512]
                )
            for c in range(KC):
                nc.sync.dma_start(out=wv_sb[c], in_=wv[c * 128:(c + 1) * 128, :])
                nc.scalar.dma_start(
                    out=xt_sb[c][:, 512:1024],
                    in_=xt[c * 128:(c + 1) * 128, 512:1024],
                )
            for c in range(KC):
                nc.sync.dma_start(
                    out=ct_sb[c][:, 512:1024],
                    in_=ct[c * 128:(c + 1) * 128, 512:1024],
                )
                nc.scalar.dma_start(out=wo_sb[c], in_=wo[c * 128:(c + 1) * 128, :])
            nc.sync.dma_start(
                out=bo_sb, in_=bo[:, :].rearrange("(c p) o -> p c o", p=128)
            )

            # ---- filler building blocks (each emits a short dense PE chain
            # plus its DVE epilogue) ----
            def proj_unit(dst, w, src, dc, ic):
                ps = paux.tile([128, 512], f32, tag="paux", name="paux")
                for kc in range(KC):
                    nc.tensor.matmul(
                        ps,
                        lhsT=w[kc][:, dc * 128:(dc + 1) * 128],
                        rhs=src[kc][:, ic * 512:(ic + 1) * 512],
                        start=(kc == 0),
                        stop=(kc == KC - 1),
                    )
                nc.scalar.copy(dst[dc][:, ic * 512:(ic + 1) * 512], ps)

            def v_unit(jc):
                ps = paux.tile([128, 512], f32, tag="paux", name="paux")
                for kc in range(KC):
                    nc.tensor.matmul(
                        ps,
                        lhsT=ct_sb[kc][:, jc * 128:(jc + 1) * 128],
                        rhs=wv_sb[kc],
                        start=(kc == 0),
                        stop=(kc == KC - 1),
                    )
                nc.vector.memset(v_sb[jc][:, :, DH:DH + 1], 1.0)
                nc.vector.tensor_copy(
                    v_sb[jc][:, :, 0:DH],
                    ps.rearrange("p (h d) -> p h d", h=H),
                )

            def out_unitA(ec, ic):
                # acc = Wo[dc0]^T @ On[dc0] + bo
                psf = paux.tile([128, 512], f32, tag="paux", name="paux")
                nc.tensor.matmul(
                    psf,
                    lhsT=wo_sb[0][:, ec * 128:(ec + 1) * 128],
                    rhs=on_sb[0][:, ic * 512:(ic + 1) * 512],
                    start=True,
                    stop=True,
                )
                nc.vector.tensor_scalar_add(
                    acc_sb[ec][:, ic * 512:(ic + 1) * 512], psf, bo_sb[:, ec, :]
                )

            def out_unitA2(ec, ic):
                # acc2 = acc + Wo[dc1,dc2]^T @ On[dc1,dc2]
                psf = paux.tile([128, 512], f32, tag="paux", name="paux")
                for dcc in (1, 2):
                    nc.tensor.matmul(
                        psf,
                        lhsT=wo_sb[dcc][:, ec * 128:(ec + 1) * 128],
                        rhs=on_sb[dcc][:, ic * 512:(ic + 1) * 512],
                        start=(dcc == 1),
                        stop=(dcc == 2),
                    )
                nc.vector.tensor_add(
                    acc2_sb[ec][:, ic * 512:(ic + 1) * 512],
                    acc_sb[ec][:, ic * 512:(ic + 1) * 512],
                    psf,
                )

            def out_unitB(ec, ic):
                psf = pB.tile([128, 512], f32, tag="pB", name="pB")
                nc.tensor.matmul(
                    psf,
                    lhsT=wo_sb[3][:, ec * 128:(ec + 1) * 128],
                    rhs=on_sb[3][:, ic * 512:(ic + 1) * 512],
                    start=True,
                    stop=True,
                )
                ot = otp.tile([128, 512], f32, tag="ot", name="ot")
                nc.vector.tensor_add(
                    ot, acc2_sb[ec][:, ic * 512:(ic + 1) * 512], psf
                )
                q = nc.sync if (ec + ic) % 2 == 0 else nc.scalar
                q.dma_start(
                    out=outT[ec * 128:(ec + 1) * 128, ic * 512:(ic + 1) * 512],
                    in_=ot,
                )

            def normalize(h):
                dc = h // 2
                pb = (h % 2) * 64
                pso = psos[h]
                lst = ofp.tile([1, N], f32, tag="of", name="of")
                linv = lp.tile([1, N], f32, tag="lv", name="lv")
                r = rp.tile([128, N], f32, tag="rb", name="rb")
                if h < H - 1:
                    nc.vector.tensor_copy(o_sb[dc][pb:pb + 64, :], pso[0:DH, :])
                    nc.vector.tensor_copy(lst, pso[DH:DH + 1, :])
                    nc.vector.reciprocal_approx_fast(out=linv, in_=lst)
                    nc.gpsimd.partition_broadcast(r, linv[0:1, :])
                    nc.vector.tensor_mul(
                        on_sb[dc][pb:pb + 64, :], o_sb[dc][pb:pb + 64, :],
                        r[pb:pb + 64, :]
                    )
                else:
                    # tail: normalize in 512-column halves (each half of pso
                    # is a separate accumulation chain, complete at its own
                    # stop), then the last out-projection pass + store. The
                    # unitB DVE adds are emitted after both halves so they
                    # don't block the second chain in the in-order queue.
                    for ic in range(IC):
                        cs = slice(ic * 512, (ic + 1) * 512)
                        nc.vector.tensor_copy(
                            o_sb[dc][pb:pb + 64, cs], pso[0:DH, cs]
                        )
                        nc.vector.tensor_copy(lst[:, cs], pso[DH:DH + 1, cs])
                        nc.vector.reciprocal_approx_fast(
                            out=linv[:, cs], in_=lst[:, cs]
                        )
                        nc.gpsimd.partition_broadcast(r[:, cs], linv[0:1, cs])
                        nc.vector.tensor_mul(
                            on_sb[dc][pb:pb + 64, cs],
                            o_sb[dc][pb:pb + 64, cs],
                            r[pb:pb + 64, cs],
                        )

            # ---- HAM warm-up: a dense burst of throwaway matmuls on
            # zeroed scratch while the input DMAs are still in flight, so
            # the PE clock gate is already at 2.4 GHz when real data lands
            # (cold matmuls run at 1.2 GHz for the first ~3.4us of
            # activity; the PE would otherwise sit idle here) ----
            scratch = sb([128, 512], bf16, "scr")
            nc.vector.memset(scratch, 0.0)

            def warm(n):
                for _ in range(n):
                    psd = paux.tile([128, 512], f32, tag="paux", name="paux")
                    nc.tensor.matmul(
                        psd, lhsT=scratch[:, 0:128], rhs=scratch,
                        start=True, stop=True,
                    )

            # ---- pre-phase: only what the first S/exp strictly needs; V
            # and the rest stream in as fillers during head 0. Dummy bursts
            # bridge the DMA-arrival gaps so the clock gate never sees an
            # idle window. ----
            warm(12)
            proj_unit(qt_sb, wq_sb, xt_sb, 0, 0)
            warm(8)
            proj_unit(kt_sb, wk_sb, ct_sb, 0, 0)
            warm(8)
            proj_unit(qt_sb, wq_sb, xt_sb, 0, 1)

            # ---- filler schedule: iteration index (h*8+jc of the S side)
            # -> closures. V[jc] must land before attn@V[0,jc] (one
            # iteration behind S); QT/KT[dc] before head 2*dc; the
            # out-projection passes trail the head-pair normalizations
            # they consume. ----
            fill = {}

            def at(it, fn, *args):
                fill.setdefault(it, []).append((fn, args))

            at(1, v_unit, 0)
            at(1, v_unit, 1)
            at(2, v_unit, 2)
            at(2, v_unit, 3)
            at(3, proj_unit, kt_sb, wk_sb, ct_sb, 0, 1)
            at(3, v_unit, 4)
            at(4, v_unit, 5)
            at(4, v_unit, 6)
            at(5, v_unit, 7)
            at(8, proj_unit, qt_sb, wq_sb, xt_sb, 1, 0)
            at(10, proj_unit, qt_sb, wq_sb, xt_sb, 1, 1)
            at(12, proj_unit, kt_sb, wk_sb, ct_sb, 1, 0)
            at(14, proj_unit, kt_sb, wk_sb, ct_sb, 1, 1)
            at(17, proj_unit, qt_sb, wq_sb, xt_sb, 2, 0)
            at(20, proj_unit, qt_sb, wq_sb, xt_sb, 2, 1)
            at(23, proj_unit, kt_sb, wk_sb, ct_sb, 2, 0)
            at(26, proj_unit, kt_sb, wk_sb, ct_sb, 2, 1)
            at(29, proj_unit, qt_sb, wq_sb, xt_sb, 3, 0)
            at(32, proj_unit, qt_sb, wq_sb, xt_sb, 3, 1)
            at(35, proj_unit, kt_sb, wk_sb, ct_sb, 3, 0)
            at(38, proj_unit, kt_sb, wk_sb, ct_sb, 3, 1)
            for i, (ec, ic) in enumerate(
                [(e, i) for e in range(EC) for i in range(IC)]
            ):
                at(41 + i, out_unitA, ec, ic)    # needs on_sb[0] (head pair 0)
                at(56 + i, out_unitA2, ec, ic)   # needs on_sb[1], on_sb[2];
                # placed in head 7 so the PE stays dense (and warm) right up
                # to the tail

            # ---- attention, software-pipelined: S/exp for iteration k are
            # emitted before attn@V for iteration k-1, so the in-order PE
            # never has an exp-dependent matmul blocking the next S. The
            # psS/psO pools close after the last head so the tail pass gets
            # six PSUM banks of its own. ----
            psos = {}
            ptiles = {}
            attn_pools = tc.tile_pool(name="psS", bufs=2, space="PSUM")
            psS = attn_pools.__enter__()
            psO_cm = tc.tile_pool(name="psO", bufs=1, space="PSUM")
            psO = psO_cm.__enter__()
            for git in range(H * JC + 1):
                if git < H * JC:
                    h, jc = divmod(git, JC)
                    dc = h // 2
                    pb = (h % 2) * 64
                    pss = psS.tile([128, N], f32, tag="psS", name="psS")
                    for ic in range(IC):
                        nc.tensor.matmul(
                            pss[:, ic * 512:(ic + 1) * 512],
                            lhsT=kt_sb[dc][pb:pb + 64, jc * 128:(jc + 1) * 128],
                            rhs=qt_sb[dc][pb:pb + 64, ic * 512:(ic + 1) * 512],
                            start=True,
                            stop=True,
                        )
                    ptile = ptp.tile([128, N], bf16, tag="pt", name="pt")
                    nc.scalar.activation(out=ptile, in_=pss, func=Exp, scale=0.125)
                    ptiles[git] = ptile
                    # fillers after the exp: their matmuls pad the PE while
                    # the exp runs, and their scalar-queue copies can't
                    # delay this iteration's exp
                    for fn, args in fill.get(git, ()):
                        fn(*args)
                if git >= 1:
                    hp, jp = divmod(git - 1, JC)
                    if jp == 0:
                        psos[hp] = psO.tile(
                            [DH + 1, N], f32, tag="psO", name="psO"
                        )
                    ptile = ptiles.pop(git - 1)
                    for ic in range(IC):
                        nc.tensor.matmul(
                            psos[hp][:, ic * 512:(ic + 1) * 512],
                            lhsT=v_sb[jp][:, hp, :],
                            rhs=ptile[:, ic * 512:(ic + 1) * 512],
                            start=(jp == 0),
                            stop=(jp == JC - 1),
                        )
                    if jp == JC - 1:
                        normalize(hp)
            psO_cm.__exit__(None, None, None)
            attn_pools.__exit__(None, None, None)

            # ---- tail: last out-projection pass with deep PSUM buffering.
            # A dummy burst keeps the clock gate warm while the final
            # normalize chain runs on the DVE. ----
            warm(10)
            with tc.tile_pool(name="pB", bufs=6, space="PSUM") as pB:
                for ic in range(IC):
                    for ec in range(EC):
                        out_unitB(ec, ic)

    nc.finalize()
    return nc


def _ensure_ntff_hook():
    """Install antenv.axon_hooks if the image lacks it, registering the
    ctypes NTFF-profile hook against libaxon_pjrt.so. Without this,
    run_bass_kernel_spmd(trace=True)/BASS_TRACE=1 crashes on import."""
    import contextlib
    import ctypes
    import os
    import sys
    import types

    try:
        import antenv.axon_hooks  # noqa: F401
        return
    except ImportError:
        pass
    try:
        import antenv
    except ImportError:
        return

    state = {"hook": None}
    mod = types.ModuleType("antenv.axon_hooks")
    mod.set_axon_ntff_profile_hook = lambda h: state.__setitem__("hook", h)
    mod.get_axon_ntff_profile_hook = lambda: state["hook"]
    sys.modules["antenv.axon_hooks"] = mod
    antenv.axon_hooks = mod

    so_path = "/opt/axon/libaxon_pjrt.so"
    if not os.path.exists(so_path):
        return
    try:
        lib = ctypes.CDLL(so_path)
    except OSError:
        return
    if not hasattr(lib, "axon_start_nrt_profile"):
        return
    lib.axon_start_nrt_profile.argtypes = [
        ctypes.POINTER(ctypes.c_int64), ctypes.c_size_t,
    ]
    lib.axon_start_nrt_profile.restype = ctypes.c_int64
    lib.axon_stop_nrt_profile.argtypes = [ctypes.c_char_p]
    lib.axon_stop_nrt_profile.restype = ctypes.c_int64

    @contextlib.contextmanager
    def _hook(output_dir, device_ids):
        import jax
        jax.devices()  # force PJRT init so the .so's client exists
        if device_ids:
            ids = (ctypes.c_int64 * len(device_ids))(*device_ids)
            rc = lib.axon_start_nrt_profile(ids, len(device_ids))
        else:
            rc = lib.axon_start_nrt_profile(None, 0)
        if rc != 0:
            raise RuntimeError(f"axon_start_nrt_profile rc={rc}")
        try:
            yield
        finally:
            n = lib.axon_stop_nrt_profile(str(output_dir).encode())
            if n <= 0:
                print(f"ntff profile: rc={n} (no profile output)")

    state["hook"] = _hook


def kernel(x, context, Wq, Wkv, Wo, bo):
    global LAST_RUN
    _ensure_ntff_hook()
    from concourse import bass_utils

    if "nc" not in _CACHE:
        _CACHE["nc"] = _build_nc()
    nc = _CACHE["nc"]

    wq = np.ascontiguousarray(Wq, dtype=np.float32).astype(_BF16)
    wk = np.ascontiguousarray(Wkv[:, :D], dtype=np.float32).astype(_BF16)
    wv = np.ascontiguousarray(Wkv[:, D:], dtype=np.float32).astype(_BF16)
    wo = np.ascontiguousarray(Wo, dtype=np.float32).astype(_BF16)
    bo_ = np.ascontiguousarray(np.asarray(bo, dtype=np.float32).reshape(D, 1))

    in_maps = []
    for b in range(B):
        in_maps.append({
            "xt": np.ascontiguousarray(np.asarray(x[b], np.float32).T).astype(_BF16),
            "ct": np.ascontiguousarray(np.asarray(context[b], np.float32).T).astype(_BF16),
            "wq": wq, "wk": wk, "wv": wv, "wo": wo,
            "bo": bo_,
        })

    LAST_RUN = bass_utils.run_bass_kernel_spmd(nc, in_maps, list(range(N_CORES)))
    out = np.empty((B, N, D), dtype=np.float32)
    for b in range(B):
        out[b] = LAST_RUN.results[b]["outT"].T
    return out


# revision 28
# speedup vs baseline: 1.1635x; 1.1635x over previous
"""Trainium2 Bass kernel: batched multi-head cross-attention.

Reference computation (per batch element b):
    q = x @ Wq; k,v = split(context @ Wkv)
    per head: attn = softmax(q k^T / 8); o = attn @ v
    out = concat_heads(o) @ Wo + bo

Sharding: pure data parallel - batch B=8, one batch element per NeuronCore,
no collectives. Inside each core a fully "transposed" dataflow (host feeds
x^T / context^T) avoids all on-device transposes:

    QT[d,i]  = Wq^T  @ xT
    KT[d,j]  = Wk^T  @ cT
    V[j,d]   = cT^T  @ Wv          (with a fused ones-column so the attn@V
                                    matmul also emits the softmax denominator)
    ST[j,i]  = KT_h^T @ QT_h       (per head, contraction d=64)
    PT       = exp(ST / 8)         (scores are O(6) so exp is fp32-safe and
                                    softmax is shift-invariant)
    OunT,l   = [V_h | 1]^T @ PT
    OnT      = OunT * (1/l)
    outT     = Wo^T @ OnT + bo

Scheduling: the PE HAM clock gate throttles the tensor engine to 1.2 GHz
whenever it sees idle windows, and the naive phase-separated schedule lets
the S->exp->O dependency chain starve the PE during attention (then the
half-clock PE *looks* saturated and never re-warms). To keep the PE stream
dense end-to-end, the QT/KT projections for later head-chunks and the
output-projection accumulation are emitted as *filler* matmul work inside
the attention loop; the PE's 64-deep reorder window picks whichever queued
matmul is ready. Out-projection partials accumulate into SBUF across head
pairs so PSUM stays within 8 banks:
    psS [128,1024] x2 (4 banks) + psO [65,1024] x1 (2) + paux [128,512] x2 (2).
"""

import os

import numpy as np
import ml_dtypes

_FILL_AFTER = os.environ.get("K_FILL_AFTER", "1") == "1"

B, N, M, D = 8, 1024, 1024, 512
H, DH = 8, 64
KC = 4          # 512 contraction -> 4 chunks of 128
IC = 2          # 1024 free dim -> 2 chunks of 512
JC = 8          # 1024 keys -> 8 chunks of 128
EC = 4          # 512 output dim -> 4 chunks of 128
N_CORES = 8

_BF16 = ml_dtypes.bfloat16
_CACHE = {}
LAST_RUN = None  # BassKernelResults of the most recent launch (for test.py)


def _build_nc():
    import concourse.bass as bass
    import concourse.mybir as mybir
    import concourse.tile as tile
    from concourse import bacc

    f32 = mybir.dt.float32
    bf16 = mybir.dt.bfloat16
    Exp = mybir.ActivationFunctionType.Exp

    nc = bacc.Bacc()

    xt = nc.declare_dram_parameter("xt", [D, N], bf16, isOutput=False)
    ct = nc.declare_dram_parameter("ct", [D, M], bf16, isOutput=False)
    wq = nc.declare_dram_parameter("wq", [D, D], bf16, isOutput=False)
    wk = nc.declare_dram_parameter("wk", [D, D], bf16, isOutput=False)
    wv = nc.declare_dram_parameter("wv", [D, D], bf16, isOutput=False)
    wo = nc.declare_dram_parameter("wo", [D, D], bf16, isOutput=False)
    bo = nc.declare_dram_parameter("bo", [D, 1], f32, isOutput=False)
    outT = nc.declare_dram_parameter("outT", [D, N], f32, isOutput=True)

    with tile.TileContext(nc) as tc:
        with (
            tc.tile_pool(name="singles", bufs=1) as singles,
            tc.tile_pool(name="pt", bufs=6) as ptp,
            tc.tile_pool(name="of", bufs=2) as ofp,
            tc.tile_pool(name="lv", bufs=2) as lp,
            tc.tile_pool(name="rb", bufs=2) as rp,
            tc.tile_pool(name="ot", bufs=3) as otp,
            tc.tile_pool(name="paux", bufs=2, space="PSUM") as paux,
        ):
            def sb(shape, dt, tag):
                return singles.tile(shape, dt, tag=tag, name=tag)

            wq_sb = [sb([128, D], bf16, f"wq{c}") for c in range(KC)]
            xt_sb = [sb([128, N], bf16, f"xt{c}") for c in range(KC)]
            wk_sb = [sb([128, D], bf16, f"wk{c}") for c in range(KC)]
            ct_sb = [sb([128, M], bf16, f"ct{c}") for c in range(KC)]
            wv_sb = [sb([128, D], bf16, f"wv{c}") for c in range(KC)]
            wo_sb = [sb([128, D], bf16, f"wo{c}") for c in range(KC)]
            qt_sb = [sb([128, N], bf16, f"qt{c}") for c in range(KC)]
            kt_sb = [sb([128, M], bf16, f"kt{c}") for c in range(KC)]
            v_sb = [sb([128, H, DH + 1], bf16, f"v{j}") for j in range(JC)]
            o_sb = [sb([128, N], bf16, f"o{c}") for c in range(KC)]
            on_sb = [sb([128, N], bf16, f"on{c}") for c in range(KC)]
            acc_sb = [sb([128, N], f32, f"acc{e}") for e in range(EC)]
            acc2_sb = [sb([128, N], f32, f"ac2{e}") for e in range(EC)]
            bo_sb = sb([128, KC, 1], f32, "bo")

            # ---- loads: two HWDGE queues (sync + scalar), batches
            # ordered by when the pre-phase needs each tensor ----
            for c in range(KC):
                nc.sync.dma_start(out=wq_sb[c], in_=wq[c * 128:(c + 1) * 128, :])
                nc.scalar.dma_start(
                    out=xt_sb[c][:, 0:512], in_=xt[c * 128:(c + 1) * 128, 0:512]
                )
            for c in range(KC):
                nc.sync.dma_start(out=wk_sb[c], in_=wk[c * 128:(c + 1) * 128, :])
                nc.scalar.dma_start(
                    out=ct_sb[c][:, 0:512], in_=ct[c * 128:(c + 1) * 128, 0:512]
                )
            for c in range(KC):
                nc.sync.dma_start(out=wv_sb[c], in_=wv[c * 128:(c + 1) * 128, :])
                nc.scalar.dma_start(
                    out=xt_sb[c][:, 512:1024],
                    in_=xt[c * 128:(c + 1) * 128, 512:1024],
                )
            for c in range(KC):
                nc.sync.dma_start(
                    out=ct_sb[c][:, 512:1024],
                    in_=ct[c * 128:(c + 1) * 128, 512:1024],
                )
                nc.scalar.dma_start(out=wo_sb[c], in_=wo[c * 128:(c + 1) * 128, :])
            nc.sync.dma_start(
                out=bo_sb, in_=bo[:, :].rearrange("(c p) o -> p c o", p=128)
            )

            # ---- filler building blocks (each emits a short dense PE chain
            # plus its DVE epilogue) ----
            def proj_unit(dst, w, src, dc, ic):
                ps = paux.tile([128, 512], f32, tag="paux", name="paux")
                for kc in range(KC):
                    nc.tensor.matmul(
                        ps,
                        lhsT=w[kc][:, dc * 128:(dc + 1) * 128],
                        rhs=src[kc][:, ic * 512:(ic + 1) * 512],
                        start=(kc == 0),
                        stop=(kc == KC - 1),
                    )
                nc.scalar.copy(dst[dc][:, ic * 512:(ic + 1) * 512], ps)

            def v_unit(jc):
                ps = paux.tile([128, 512], f32, tag="paux", name="paux")
                for kc in range(KC):
                    nc.tensor.matmul(
                        ps,
                        lhsT=ct_sb[kc][:, jc * 128:(jc + 1) * 128],
                        rhs=wv_sb[kc],
                        start=(kc == 0),
                        stop=(kc == KC - 1),
                    )
                nc.vector.memset(v_sb[jc][:, :, DH:DH + 1], 1.0)
                nc.vector.tensor_copy(
                    v_sb[jc][:, :, 0:DH],
                    ps.rearrange("p (h d) -> p h d", h=H),
                )

            def out_unitA(ec, ic):
                # acc = Wo[dc0]^T @ On[dc0] + bo
                psf = paux.tile([128, 512], f32, tag="paux", name="paux")
                nc.tensor.matmul(
                    psf,
                    lhsT=wo_sb[0][:, ec * 128:(ec + 1) * 128],
                    rhs=on_sb[0][:, ic * 512:(ic + 1) * 512],
                    start=True,
                    stop=True,
                )
                nc.vector.tensor_scalar_add(
                    acc_sb[ec][:, ic * 512:(ic + 1) * 512], psf, bo_sb[:, ec, :]
                )

            def out_unitA2(ec, ic):
                # acc2 = acc + Wo[dc1,dc2]^T @ On[dc1,dc2]
                psf = paux.tile([128, 512], f32, tag="paux", name="paux")
                for dcc in (1, 2):
                    nc.tensor.matmul(
                        psf,
                        lhsT=wo_sb[dcc][:, ec * 128:(ec + 1) * 128],
                        rhs=on_sb[dcc][:, ic * 512:(ic + 1) * 512],
                        start=(dcc == 1),
                        stop=(dcc == 2),
                    )
                nc.vector.tensor_add(
                    acc2_sb[ec][:, ic * 512:(ic + 1) * 512],
                    acc_sb[ec][:, ic * 512:(ic + 1) * 512],
                    psf,
                )

            def out_unitB(ec, ic):
                psf = pB.tile([128, 512], f32, tag="pB", name="pB")
                nc.tensor.matmul(
                    psf,
                    lhsT=wo_sb[3][:, ec * 128:(ec + 1) * 128],
                    rhs=on_sb[3][:, ic * 512:(ic + 1) * 512],
                    start=True,
                    stop=True,
                )
                ot = otp.tile([128, 512], f32, tag="ot", name="ot")
                nc.vector.tensor_add(
                    ot, acc2_sb[ec][:, ic * 512:(ic + 1) * 512], psf
                )
                q = nc.sync if (ec + ic) % 2 == 0 else nc.scalar
                q.dma_start(
                    out=outT[ec * 128:(ec + 1) * 128, ic * 512:(ic + 1) * 512],
                    in_=ot,
                )

            def normalize(h):
                dc = h // 2
                pb = (h % 2) * 64
                pso = psos[h]
                lst = ofp.tile([1, N], f32, tag="of", name="of")
                linv = lp.tile([1, N], f32, tag="lv", name="lv")
                r = rp.tile([128, N], f32, tag="rb", name="rb")
                if h < H - 1:
                    nc.vector.tensor_copy(o_sb[dc][pb:pb + 64, :], pso[0:DH, :])
                    nc.vector.tensor_copy(lst, pso[DH:DH + 1, :])
                    nc.vector.reciprocal_approx_fast(out=linv, in_=lst)
                    nc.gpsimd.partition_broadcast(r, linv[0:1, :])
                    nc.vector.tensor_mul(
                        on_sb[dc][pb:pb + 64, :], o_sb[dc][pb:pb + 64, :],
                        r[pb:pb + 64, :]
                    )
                else:
                    # tail: normalize in 512-column halves (each half of pso
                    # is a separate accumulation chain, complete at its own
                    # stop), then the last out-projection pass + store. The
                    # unitB DVE adds are emitted after both halves so they
                    # don't block the second chain in the in-order queue.
                    for ic in range(IC):
                        cs = slice(ic * 512, (ic + 1) * 512)
                        nc.vector.tensor_copy(
                            o_sb[dc][pb:pb + 64, cs], pso[0:DH, cs]
                        )
                        nc.vector.tensor_copy(lst[:, cs], pso[DH:DH + 1, cs])
                        nc.vector.reciprocal_approx_fast(
                            out=linv[:, cs], in_=lst[:, cs]
                        )
                        nc.gpsimd.partition_broadcast(r[:, cs], linv[0:1, cs])
                        nc.vector.tensor_mul(
                            on_sb[dc][pb:pb + 64, cs],
                            o_sb[dc][pb:pb + 64, cs],
                            r[pb:pb + 64, cs],
                        )

            # ---- HAM warm-up: a dense burst of throwaway matmuls on
            # zeroed scratch while the input DMAs are still in flight, so
            # the PE clock gate is already at 2.4 GHz when real data lands
            # (cold matmuls run at 1.2 GHz for the first ~3.4us of
            # activity; the PE would otherwise sit idle here) ----
            scratch = sb([128, 512], bf16, "scr")
            nc.vector.memset(scratch, 0.0)

            def warm(n):
                for _ in range(n):
                    psd = paux.tile([128, 512], f32, tag="paux", name="paux")
                    nc.tensor.matmul(
                        psd, lhsT=scratch[:, 0:128], rhs=scratch,
                        start=True, stop=True,
                    )

            # ---- pre-phase: only what the first S/exp strictly needs; V
            # and the rest stream in as fillers during head 0. Dummy bursts
            # bridge the DMA-arrival gaps so the clock gate never sees an
            # idle window. ----
            warm(12)
            proj_unit(qt_sb, wq_sb, xt_sb, 0, 0)
            warm(8)
            proj_unit(kt_sb, wk_sb, ct_sb, 0, 0)
            warm(8)
            proj_unit(qt_sb, wq_sb, xt_sb, 0, 1)

            # ---- filler schedule: iteration index (h*8+jc of the S side)
            # -> closures. V[jc] must land before attn@V[0,jc] (one
            # iteration behind S); QT/KT[dc] before head 2*dc; the
            # out-projection passes trail the head-pair normalizations
            # they consume. ----
            fill = {}

            def at(it, fn, *args):
                fill.setdefault(it, []).append((fn, args))

            at(1, v_unit, 0)
            at(1, v_unit, 1)
            at(2, v_unit, 2)
            at(2, v_unit, 3)
            at(3, proj_unit, kt_sb, wk_sb, ct_sb, 0, 1)
            at(3, v_unit, 4)
            at(4, v_unit, 5)
            at(4, v_unit, 6)
            at(5, v_unit, 7)
            at(8, proj_unit, qt_sb, wq_sb, xt_sb, 1, 0)
            at(10, proj_unit, qt_sb, wq_sb, xt_sb, 1, 1)
            at(12, proj_unit, kt_sb, wk_sb, ct_sb, 1, 0)
            at(14, proj_unit, kt_sb, wk_sb, ct_sb, 1, 1)
            at(17, proj_unit, qt_sb, wq_sb, xt_sb, 2, 0)
            at(20, proj_unit, qt_sb, wq_sb, xt_sb, 2, 1)
            at(23, proj_unit, kt_sb, wk_sb, ct_sb, 2, 0)
            at(26, proj_unit, kt_sb, wk_sb, ct_sb, 2, 1)
            at(29, proj_unit, qt_sb, wq_sb, xt_sb, 3, 0)
            at(32, proj_unit, qt_sb, wq_sb, xt_sb, 3, 1)
            at(35, proj_unit, kt_sb, wk_sb, ct_sb, 3, 0)
            at(38, proj_unit, kt_sb, wk_sb, ct_sb, 3, 1)
            for i, (ec, ic) in enumerate(
                [(e, i) for e in range(EC) for i in range(IC)]
            ):
                at(41 + i, out_unitA, ec, ic)    # needs on_sb[0] (head pair 0)
                at(56 + i, out_unitA2, ec, ic)   # needs on_sb[1], on_sb[2];
                # placed in head 7 so the PE stays dense (and warm) right up
                # to the tail

            # ---- attention, software-pipelined: S/exp for iteration k are
            # emitted before attn@V for iteration k-1, so the in-order PE
            # never has an exp-dependent matmul blocking the next S. The
            # psS/psO pools close after the last head so the tail pass gets
            # six PSUM banks of its own. ----
            psos = {}
            ptiles = {}
            attn_pools = tc.tile_pool(name="psS", bufs=2, space="PSUM")
            psS = attn_pools.__enter__()
            psO_cm = tc.tile_pool(name="psO", bufs=1, space="PSUM")
            psO = psO_cm.__enter__()
            for git in range(H * JC + 1):
                if git < H * JC:
                    h, jc = divmod(git, JC)
                    dc = h // 2
                    pb = (h % 2) * 64
                    if not _FILL_AFTER:
                        for fn, args in fill.get(git, ()):
                            fn(*args)
                    pss = psS.tile([128, N], f32, tag="psS", name="psS")
                    for ic in range(IC):
                        nc.tensor.matmul(
                            pss[:, ic * 512:(ic + 1) * 512],
                            lhsT=kt_sb[dc][pb:pb + 64, jc * 128:(jc + 1) * 128],
                            rhs=qt_sb[dc][pb:pb + 64, ic * 512:(ic + 1) * 512],
                            start=True,
                            stop=True,
                        )
                    ptile = ptp.tile([128, N], bf16, tag="pt", name="pt")
                    nc.scalar.activation(out=ptile, in_=pss, func=Exp, scale=0.125)
                    ptiles[git] = ptile
                    if _FILL_AFTER:
                        # fillers after the exp: their scalar-queue copies
                        # can't delay this iteration's exp
                        for fn, args in fill.get(git, ()):
                            fn(*args)
                if git >= 1:
                    hp, jp = divmod(git - 1, JC)
                    if jp == 0:
                        psos[hp] = psO.tile(
                            [DH + 1, N], f32, tag="psO", name="psO"
                        )
                    ptile = ptiles.pop(git - 1)
                    for ic in range(IC):
                        nc.tensor.matmul(
                            psos[hp][:, ic * 512:(ic + 1) * 512],
                            lhsT=v_sb[jp][:, hp, :],
                            rhs=ptile[:, ic * 512:(ic + 1) * 512],
                            start=(jp == 0),
                            stop=(jp == JC - 1),
                        )
                    if jp == JC - 1:
                        normalize(hp)
            psO_cm.__exit__(None, None, None)
            attn_pools.__exit__(None, None, None)

            # ---- tail: last out-projection pass with deep PSUM buffering.
            # A dummy burst keeps the clock gate warm while the final
            # normalize chain runs on the DVE. ----
            warm(10)
            with tc.tile_pool(name="pB", bufs=6, space="PSUM") as pB:
                for ic in range(IC):
                    for ec in range(EC):
                        out_unitB(ec, ic)

    nc.finalize()
    return nc


def _ensure_ntff_hook():
    """Install antenv.axon_hooks if the image lacks it, registering the
    ctypes NTFF-profile hook against libaxon_pjrt.so. Without this,
    run_bass_kernel_spmd(trace=True)/BASS_TRACE=1 crashes on import."""
    import contextlib
    import ctypes
    import os
    import sys
    import types

    try:
        import antenv.axon_hooks  # noqa: F401
        return
    except ImportError:
        pass
    try:
        import antenv
    except ImportError:
        return

    state = {"hook": None}
    mod = types.ModuleType("antenv.axon_hooks")
    mod.set_axon_ntff_profile_hook = lambda h: state.__setitem__("hook", h)
    mod.get_axon_ntff_profile_hook = lambda: state["hook"]
    sys.modules["antenv.axon_hooks"] = mod
    antenv.axon_hooks = mod

    so_path = "/opt/axon/libaxon_pjrt.so"
    if not os.path.exists(so_path):
        return
    try:
        lib = ctypes.CDLL(so_path)
    except OSError:
        return
    if not hasattr(lib, "axon_start_nrt_profile"):
        return
    lib.axon_start_nrt_profile.argtypes = [
        ctypes.POINTER(ctypes.c_int64), ctypes.c_size_t,
    ]
    lib.axon_start_nrt_profile.restype = ctypes.c_int64
    lib.axon_stop_nrt_profile.argtypes = [ctypes.c_char_p]
    lib.axon_stop_nrt_profile.restype = ctypes.c_int64

    @contextlib.contextmanager
    def _hook(output_dir, device_ids):
        import jax
        jax.devices()  # force PJRT init so the .so's client exists
        if device_ids:
            ids = (ctypes.c_int64 * len(device_ids))(*device_ids)
            rc = lib.axon_start_nrt_profile(ids, len(device_ids))
        else:
            rc = lib.axon_start_nrt_profile(None, 0)
        if rc != 0:
            raise RuntimeError(f"axon_start_nrt_profile rc={rc}")
        try:
            yield
        finally:
            n = lib.axon_stop_nrt_profile(str(output_dir).encode())
            if n <= 0:
                print(f"ntff profile: rc={n} (no profile output)")

    state["hook"] = _hook


def kernel(x, context, Wq, Wkv, Wo, bo):
    global LAST_RUN
    _ensure_ntff_hook()
    from concourse import bass_utils

    if "nc" not in _CACHE:
        _CACHE["nc"] = _build_nc()
    nc = _CACHE["nc"]

    wq = np.ascontiguousarray(Wq, dtype=np.float32).astype(_BF16)
    wk = np.ascontiguousarray(Wkv[:, :D], dtype=np.float32).astype(_BF16)
    wv = np.ascontiguousarray(Wkv[:, D:], dtype=np.float32).astype(_BF16)
    wo = np.ascontiguousarray(Wo, dtype=np.float32).astype(_BF16)
    bo_ = np.ascontiguousarray(np.asarray(bo, dtype=np.float32).reshape(D, 1))

    in_maps = []
    for b in range(B):
        in_maps.append({
            "xt": np.ascontiguousarray(np.asarray(x[b], np.float32).T).astype(_BF16),
            "ct": np.ascontiguousarray(np.asarray(context[b], np.float32).T).astype(_BF16),
            "wq": wq, "wk": wk, "wv": wv, "wo": wo,
            "bo": bo_,
        })

    LAST_RUN = bass_utils.run_bass_kernel_spmd(nc, in_maps, list(range(N_CORES)))
    out = np.empty((B, N, D), dtype=np.float32)
    for b in range(B):
        out[b] = LAST_RUN.results[b]["outT"].T
    return out
